# revision 31
# baseline (speedup 1.0000x reference)
"""NonLocalAttention (embedded gaussian, no softmax) on 8 trn2 NeuronCores.

Reference math (per sample, all linear — no softmax):
    theta = conv1x1(a, theta_w, theta_b)        # [Ci, N]
    phi   = conv1x1(b, phi_w, phi_b)            # [Ci, N]
    g     = conv1x1(b, g_w, g_b)                # [Ci, N]
    f     = theta^T @ phi / N                   # [N, N]
    y     = f @ g^T                             # [N, Ci]
    out   = BN(W_w @ y^T)                       # [C, N]

Everything is linear, so the whole network collapses to a per-sample
256x256 channel-mixing matrix applied to `a`:
    Mi[ci1, ci2] = sum_m phi[ci1, m] * g[ci2, m]          # [128, 128]
    R^T = Mi-contract W'^T  (W' = bn_scale * W_w)         # [128, 256]
    ta  = theta'^T-contract a  (theta' = theta_w/N)       # [128, N]
    out = R^T-contract ta + shift                         # [256, N]

Mi is produced without PE transposes: for each 128-pixel chunk of b,
matmul(lhsT=b_chunk, rhs=[phiT|gT]) directly yields phi^T/g^T tiles with
pixels on partitions, which feed the Mi contraction.

All activations and weights move to the device as bf16 (halves HBM
traffic); accumulation stays f32 in PSUM; output returns as bf16 and is
cast to f32 on the host. Biases (zero in this problem, but handled
exactly): phi_b/g_b fold into a host-computed rank-2 correction to Mi
(needs only rowsums of b); theta_b is the bias of the ta eviction; BN
scale folds into W^T on the host.

Sharding: 8 cores = 4 samples x 2 pixel-halves of `a`. Each core loads
the full per-sample b (Mi is duplicated across the pair — cheaper than
any cross-core exchange) and its half of a; no inter-core communication.
"""

import numpy as np

B, C, Ci, H, W = 4, 256, 128, 64, 64
N_PIX = H * W            # 4096 pixels per sample
N_CORES = 8
HALF = N_PIX // 2        # 2048 output pixels per core
P = 128
CC = C // P              # 2 channel chunks
NCH = 8                  # b DMA chunks (512 px each) == phase-1 quads
QPIX = N_PIX // NCH      # 512 pixels per chunk/quad
RB = 512                 # output row block
BN_EPS = 1e-5

WARMUP_MM = 18           # junk matmuls to lift the PE HAM throttle early

# wpack column layout (bf16, partition dim = 128):
#   [0,256)     cc0: [phiT | gT]    [c_in_chunk, ci]
#   [256,512)   cc1: [phiT | gT]
#   [512,768)   (theta_w/N)^T      [c (2 chunks), ci1]
#   [768,1024)  (W_w * bn_scale)^T [ci2, c_out]
WCOLS = 1024
# vpack (f32): [0,2) bn shift per cc, [2] theta_b/N, [3,131) Mi correction
VCOLS = 131

_CACHE = {}


def _build():
    import concourse.bacc as bacc
    import concourse.mybir as mybir
    import concourse.tile as tile
    from concourse.masks import make_identity

    f32 = mybir.dt.float32
    bf16 = mybir.dt.bfloat16
    Act = mybir.ActivationFunctionType

    nc = bacc.Bacc("TRN2", num_devices=N_CORES)

    wb0_d = nc.dram_tensor("wb0", [P, 512 + CC * QPIX], bf16,
                           kind="ExternalInput")
    wpackB_d = nc.dram_tensor("wpackB", [P, WCOLS - 512], bf16,
                              kind="ExternalInput")
    vpack_d = nc.dram_tensor("vpack", [P, VCOLS], f32, kind="ExternalInput")
    a_d = nc.dram_tensor("a_half", [CC, P, HALF], bf16, kind="ExternalInput")
    b_d = nc.dram_tensor("b_rest", [NCH - 1, CC, P, QPIX], bf16,
                         kind="ExternalInput")
    out_d = nc.dram_tensor("out", [CC, P, HALF], bf16, kind="ExternalOutput")

    with tile.TileContext(nc) as tc:
        with (
            tc.tile_pool(name="const", bufs=1) as cpool,
            tc.tile_pool(name="big", bufs=1) as bpool,
            tc.tile_pool(name="work", bufs=2) as wpool,
            tc.tile_pool(name="ps", bufs=3, space="PSUM") as ppool,
        ):
            # wb_sb: [conv weights (512) | b chunk-major: chunk q at
            # 512 + q*CC*QPIX, cc-halves of QPIX pixels each]
            wb_sb = bpool.tile([P, 512 + CC * N_PIX], bf16)
            wpackB_sb = cpool.tile([P, WCOLS - 512], bf16)
            vpack_sb = cpool.tile([P, VCOLS], f32)
            a_sb = bpool.tile([P, CC, HALF], bf16)

            conv_w = wb_sb[:, 0:512].rearrange("p (c k) -> p c k", c=CC)
            thwT = wpackB_sb[:, 0:256].rearrange("p (c k) -> p c k", c=CC)
            WT_sb = wpackB_sb[:, 256:512]
            shift_in = vpack_sb[:, 0:2]
            thb_sb = vpack_sb[:, 2:3]
            cmi_sb = vpack_sb[:, 3:131]

            def b_chunk(m, cc):
                # 128-px chunk m, channel half cc -> [P, 128] slice
                col = 512 + (m // 4) * CC * QPIX + cc * QPIX + (m % 4) * P
                return wb_sb[:, col : col + P]

            # single SP FIFO keeps the transfer order exactly as needed:
            # conv weights fused with b chunk 0, remaining b chunks,
            # phase-2/3 consts, a (only needed by phase 3), then stores.
            nc.sync.dma_start(out=wb_sb[:, 0 : 512 + CC * QPIX],
                              in_=wb0_d[:])
            for q in range(1, NCH):
                lo = 512 + q * CC * QPIX
                nc.sync.dma_start(
                    out=wb_sb[:, lo : lo + CC * QPIX].rearrange(
                        "p (c x) -> p c x", c=CC),
                    in_=b_d[q - 1].rearrange("c p x -> p c x"),
                )
            nc.sync.dma_start(out=wpackB_sb[:], in_=wpackB_d[:])
            nc.sync.dma_start(out=vpack_sb[:], in_=vpack_d[:])
            nc.sync.dma_start(out=a_sb[:], in_=a_d.rearrange("c p x -> p c x"))

            # ---- engine warmup ------------------------------------------
            # Touch the scalar engine immediately so its activation-table
            # load (1.3us) runs during the initial DMA wait, not in front of
            # the first phase-1 eviction.
            act_warm = cpool.tile([P, 8], f32)
            nc.scalar.memzero(act_warm[:, 0:4])
            nc.scalar.copy(act_warm[:, 4:8], act_warm[:, 0:4])

            # ---- PE warmup: sustained matmuls on a gpsimd-built tile so
            # the HAM clock gate lifts before the real convs arrive (needs
            # no DMA — runs from t~0 while the inputs stream in).
            if WARMUP_MM:
                ident_f32 = cpool.tile([P, P], f32)
                ident_bf = cpool.tile([P, P], bf16)
                make_identity(nc, ident_f32[:])
                nc.vector.tensor_copy(ident_bf[:], ident_f32[:])
                warm_ps = ppool.tile([P, P], f32, tag="warm", bufs=1, name="warm_ps")
                for i in range(WARMUP_MM):
                    nc.tensor.matmul(
                        warm_ps[:], ident_bf[:], ident_bf[:],
                        start=True, stop=True,
                    )

            # ---- phase 1: Mi accumulation, software-pipelined by quad -----
            # quad qd = pixel chunks 4qd..4qd+3 (128 px each) = b chunk qd.
            mi_ps = ppool.tile([Ci, Ci], f32, tag="mi", bufs=1, name="mi_ps")
            q_sbs = {}

            def emit_quad(qd):
                # [pix, 2 x (phiT | gT)] lo/hi halves; fully separate PSUM
                # tiles so the DVE and ACT evictions share no dependencies.
                lo_ps = ppool.tile([P, 2, 256], f32, tag="plo", name=f"lops{qd}")
                hi_ps = ppool.tile([P, 2, 256], f32, tag="phi", name=f"hips{qd}")
                for k in range(4):
                    m = 4 * qd + k
                    dst = lo_ps if k < 2 else hi_ps
                    for cc in range(CC):
                        nc.tensor.matmul(
                            dst[:, k % 2, :],
                            b_chunk(m, cc),
                            conv_w[:, cc, :],
                            start=(cc == 0), stop=(cc == CC - 1),
                        )
                qd_lo = wpool.tile([P, 2, 256], bf16, tag="qlo", bufs=3,
                                   name=f"qlo{qd}")
                qd_hi = wpool.tile([P, 2, 256], bf16, tag="qhi", bufs=3,
                                   name=f"qhi{qd}")
                nc.vector.tensor_copy(qd_lo[:], lo_ps[:])
                nc.scalar.copy(qd_hi[:], hi_ps[:])
                q_sbs[qd] = [qd_lo[:, 0, :], qd_lo[:, 1, :],
                             qd_hi[:, 0, :], qd_hi[:, 1, :]]

            # flipped: mi_ps[ci2, ci1] = Mi[ci1, ci2] (g as lhsT, phi as
            # rhs) so R^T comes out of a single matmul later.
            def emit_mi(qd, ks=(0, 1, 2, 3)):
                for k in ks:
                    ck = q_sbs[qd][k]
                    nc.tensor.matmul(
                        mi_ps[:], ck[:, P:256], ck[:, 0:P],
                        start=(qd == 0 and k == 0),
                        stop=(qd == NCH - 1 and k == 3),
                    )

            emit_quad(0)
            for qd in range(1, NCH):
                emit_quad(qd)
                if qd < NCH - 1:
                    emit_mi(qd - 1)
            emit_mi(NCH - 2)

            # ---- tail: finish Mi while ta = theta'^T a fills the PE -------
            NBLK = HALF // RB
            ta_sb = bpool.tile([Ci, HALF], bf16)
            mi_sb = bpool.tile([Ci, Ci], bf16)
            rt_sb = bpool.tile([Ci, C], bf16)

            def emit_ta(t):
                rows = slice(t * RB, (t + 1) * RB)
                ta_ps = ppool.tile([Ci, RB], f32,
                                   tag=("plo" if t % 2 else "phi"),
                                   name=f"taps{t}")
                for cc in range(CC):
                    nc.tensor.matmul(ta_ps[:], thwT[:, cc, :],
                                     a_sb[:, cc, rows],
                                     start=(cc == 0), stop=(cc == CC - 1))
                if t < 3:
                    nc.scalar.activation(ta_sb[:, rows], ta_ps[:],
                                         Act.Identity, bias=thb_sb)
                else:
                    nc.vector.tensor_tensor(
                        ta_sb[:, rows], ta_ps[:],
                        thb_sb.broadcast_to([Ci, RB]),
                        op=mybir.AluOpType.add)

            emit_ta(0)
            emit_mi(NCH - 1, (0, 1))
            emit_ta(1)
            emit_mi(NCH - 1, (2, 3))
            nc.vector.tensor_tensor(mi_sb[:], mi_ps[:], cmi_sb,
                                    op=mybir.AluOpType.add)
            emit_ta(2)
            rt_ps = ppool.tile([Ci, C], f32, tag="phi", name="rt_ps")
            nc.tensor.matmul(rt_ps[:], mi_sb[:], WT_sb[:],
                             start=True, stop=True)
            nc.vector.tensor_copy(rt_sb[:], rt_ps[:])
            emit_ta(3)

            # ---- out = R^T-contract ta, BN shift, store -------------------
            for r in range(NBLK):
                osz = RB
                rows = slice(r * RB, (r + 1) * RB)
                osb = wpool.tile([P, CC, osz], bf16, tag="osb", bufs=4,
                                 name=f"osb{r}")
                for co in range(CC):
                    o_ps = ppool.tile([P, osz], f32,
                                      tag=("plo" if co else "phi"),
                                      name=f"ops{r}{co}")
                    nc.tensor.matmul(o_ps[:], rt_sb[:, co * P : (co + 1) * P],
                                     ta_sb[:, rows], start=True, stop=True)
                    if co == 0:
                        nc.scalar.activation(osb[:, 0, :], o_ps[:],
                                             Act.Identity,
                                             bias=shift_in[:, 0:1])
                    else:
                        nc.vector.tensor_tensor(
                            osb[:, 1, :], o_ps[:],
                            shift_in[:, 1:2].broadcast_to([P, osz]),
                            op=mybir.AluOpType.add,
                        )
                nc.sync.dma_start(
                    out=out_d[:, :, rows].rearrange("c p r -> p c r"),
                    in_=osb[:],
                )

    nc.compile()
    return nc


def _get_nc():
    if "nc" not in _CACHE:
        _CACHE["nc"] = _build()
    return _CACHE["nc"]


def _prep_in_maps(a, b, theta_w, theta_b, phi_w, phi_b, g_w, g_b, W_w,
                  bn_gamma, bn_beta, bn_mean, bn_var):
    import ml_dtypes

    f = np.float32
    bf = ml_dtypes.bfloat16
    a4 = np.asarray(a, f).reshape(B, C, N_PIX)
    b4 = np.asarray(b, f).reshape(B, C, N_PIX)
    theta_w = np.asarray(theta_w, f)
    phi_w = np.asarray(phi_w, f)
    g_w = np.asarray(g_w, f)
    W_w = np.asarray(W_w, f)
    theta_b = np.asarray(theta_b, f)
    phi_b = np.asarray(phi_b, f)
    g_b = np.asarray(g_b, f)

    scale = (np.asarray(bn_gamma, f)
             / np.sqrt(np.asarray(bn_var, f) + BN_EPS)).astype(f)
    shift = (np.asarray(bn_beta, f) - np.asarray(bn_mean, f) * scale).astype(f)
    inv_n = 1.0 / np.float64(N_PIX)

    wpackA = np.zeros((P, 512), f)
    wpackA[:, 0:128] = phi_w.T[0:P]
    wpackA[:, 128:256] = g_w.T[0:P]
    wpackA[:, 256:384] = phi_w.T[P:C]
    wpackA[:, 384:512] = g_w.T[P:C]
    wpackB = np.zeros((P, WCOLS - 512), f)
    thT = (theta_w * inv_n).T                   # [C, Ci]
    wpackB[:, 0:128] = thT[0:P]
    wpackB[:, 128:256] = thT[P:C]
    wpackB[:, 256:512] = (W_w * scale[:, None]).T
    wpackA = wpackA.astype(bf)
    wpackB = np.ascontiguousarray(wpackB.astype(bf))

    # Mi bias correction from rowsums of b (exact; zero when biases are zero)
    rsb = b4.sum(axis=2)                        # [B, C]
    s_phi = rsb @ phi_w.T                       # [B, Ci]
    s_g = rsb @ g_w.T                           # [B, Ci]

    in_maps = []
    for core in range(N_CORES):
        s, h = divmod(core, 2)
        cmi = (phi_b[:, None] * s_g[s][None, :]
               + s_phi[s][:, None] * g_b[None, :]
               + N_PIX * phi_b[:, None] * g_b[None, :]).astype(f)
        vpack = np.zeros((P, VCOLS), f)
        vpack[:, 0] = shift[:P]
        vpack[:, 1] = shift[P:]
        vpack[:, 2] = theta_b * inv_n
        vpack[:, 3:131] = cmi.T
        bq = b4[s].reshape(CC, P, NCH, QPIX).transpose(2, 0, 1, 3).astype(bf)
        wb0 = np.concatenate([wpackA, bq[0, 0], bq[0, 1]], axis=1)
        in_maps.append({
            "a_half": np.ascontiguousarray(
                a4[s][:, h * HALF : (h + 1) * HALF]
                .reshape(CC, P, HALF).astype(bf)),
            "wb0": np.ascontiguousarray(wb0),
            "b_rest": np.ascontiguousarray(bq[1:]),
            "wpackB": wpackB,
            "vpack": np.ascontiguousarray(vpack),
        })
    return in_maps


def run(inputs: dict, trace: bool = False):
    from concourse.bass_utils import run_bass_kernel_spmd

    nc = _get_nc()
    in_maps = _prep_in_maps(**inputs)
    res = run_bass_kernel_spmd(nc, in_maps, list(range(N_CORES)), trace=trace)
    out = np.empty((B, C, N_PIX), np.float32)
    for core in range(N_CORES):
        s, h = divmod(core, 2)
        out[s][:, h * HALF : (h + 1) * HALF] = \
            res.results[core]["out"].reshape(C, HALF).astype(np.float32)
    return out.reshape(B, C, H, W), res


def kernel(**inputs) -> np.ndarray:
    out, _ = run(inputs, trace=False)
    return out


# revision 32
# speedup vs baseline: 1.0908x; 1.0908x over previous
"""NonLocalAttention (embedded gaussian, no softmax) on 8 trn2 NeuronCores.

Reference math (per sample, all linear — no softmax):
    theta = conv1x1(a, theta_w, theta_b)        # [Ci, N]
    phi   = conv1x1(b, phi_w, phi_b)            # [Ci, N]
    g     = conv1x1(b, g_w, g_b)                # [Ci, N]
    f     = theta^T @ phi / N                   # [N, N]
    y     = f @ g^T                             # [N, Ci]
    out   = BN(W_w @ y^T)                       # [C, N]

Everything is linear, so the whole network collapses to a per-sample
256x256 Gram matrix of b plus small weight products:
    S   = b b^T                                   # [256, 256], symmetric
    M3  = S K2,  K2 = g_w^T (bn_scale*W_w)^T      # K2 host-precomputed
    R^T = phi_w M3                                # [128, 256]
    ta  = theta'^T-contract a  (theta' = theta_w/N)
    out = R^T-contract ta + shift

S accumulates across 128-pixel chunks of the HOST-TRANSPOSED b (pixels on
partitions), matmul(lhsT=bT[:, c-half], rhs=bT): no PE transposes, no
per-chunk evictions — phase 1 is pure PE. M3 = S K2 needs S^T tiles as
stationary operands, which by symmetry are just the stored S tiles.

All activations and weights move as bf16 (halves HBM traffic), f32 PSUM
accumulation; output returns bf16 and is cast to f32 on the host. Biases
(zero in this problem, but handled exactly): phi_b/g_b fold into a
host-computed correction to R^T (needs only rowsums of b); theta_b is
the bias of the ta eviction; BN scale folds into K2 on the host.

Sharding: 8 cores = 4 samples x 2 pixel-halves of `a`. Each core loads
the full per-sample b (S is duplicated across the pair — cheaper than
any cross-core exchange) and its half of a; no inter-core communication.
"""

import numpy as np

B, C, Ci, H, W = 4, 256, 128, 64, 64
N_PIX = H * W            # 4096 pixels per sample
N_CORES = 8
HALF = N_PIX // 2        # 2048 output pixels per core
P = 128
CC = C // P              # 2 channel chunks
NCH = 8                  # b DMA chunks (512 px each)
MCH = N_PIX // P         # 32 pixel chunks for the S accumulation
NTA = 4                  # a DMA chunks == ta/out blocks
RB = 512                 # output row block
BN_EPS = 1e-5

WARMUP_MM = 18           # junk matmuls to lift the PE HAM throttle early

# wpack column layout (bf16, partition dim = 128):
#   [0,256)     phi_w^T    2 halves of [c1_half, ci1]
#   [256,768)   K2         2 halves of [c2_half, c_out(256)]
#   [768,1024)  (theta_w/N)^T  2 halves of [c_half, ci1]
WCOLS = 1024
# vpack (f32): [0,2) bn shift per cc, [2] theta_b/N, [3,259) R^T bias corr
VCOLS = 259

_CACHE = {}


def _build():
    import concourse.bacc as bacc
    import concourse.mybir as mybir
    import concourse.tile as tile
    from concourse.masks import make_identity

    f32 = mybir.dt.float32
    bf16 = mybir.dt.bfloat16
    Act = mybir.ActivationFunctionType

    nc = bacc.Bacc("TRN2", num_devices=N_CORES)

    b_d = nc.dram_tensor("bT", [NCH, 4, P, C], bf16, kind="ExternalInput")
    wpack_d = nc.dram_tensor("wpack", [P, WCOLS], bf16, kind="ExternalInput")
    vpack_d = nc.dram_tensor("vpack", [P, VCOLS], f32, kind="ExternalInput")
    a_d = nc.dram_tensor("a_half", [NTA, CC, P, HALF // NTA], bf16,
                         kind="ExternalInput")
    out_d = nc.dram_tensor("out", [CC, P, HALF], bf16, kind="ExternalOutput")

    with tile.TileContext(nc) as tc:
        with (
            tc.tile_pool(name="const", bufs=1) as cpool,
            tc.tile_pool(name="big", bufs=1) as bpool,
            tc.tile_pool(name="work", bufs=2) as wpool,
            tc.tile_pool(name="ps", bufs=4, space="PSUM") as ppool,
        ):
            bt_sb = bpool.tile([P, MCH, C], bf16)
            wpack_sb = cpool.tile([P, WCOLS], bf16)
            vpack_sb = cpool.tile([P, VCOLS], f32)
            a_sb = bpool.tile([P, CC, HALF], bf16)

            phwT = wpack_sb[:, 0:256].rearrange("p (h k) -> p h k", h=2)
            k2_sb = wpack_sb[:, 256:768].rearrange("p (h k) -> p h k", h=2)
            thwT = wpack_sb[:, 768:1024].rearrange("p (h k) -> p h k", h=2)
            shift_in = vpack_sb[:, 0:2]
            thb_sb = vpack_sb[:, 2:3]
            rtc_sb = vpack_sb[:, 3:259]

            # single SP FIFO: bT chunks first (phase 1 streams them), then
            # weights (needed mid-kernel), then a chunks (tail input).
            for q in range(NCH):
                nc.sync.dma_start(
                    out=bt_sb[:, 4 * q : 4 * q + 4, :],
                    in_=b_d[q].rearrange("k p c -> p k c"),
                )
            nc.sync.dma_start(out=wpack_sb[:], in_=wpack_d[:])
            nc.sync.dma_start(out=vpack_sb[:], in_=vpack_d[:])
            ap = HALF // NTA
            for t in range(NTA):
                nc.sync.dma_start(
                    out=a_sb[:, :, t * ap : (t + 1) * ap],
                    in_=a_d[t].rearrange("c p x -> p c x"),
                )

            # ---- engine warmup ------------------------------------------
            # Touch the scalar engine immediately so its activation-table
            # load (1.3us) runs during the initial DMA wait, not in front of
            # the first eviction.
            act_warm = cpool.tile([P, 8], f32)
            nc.scalar.memzero(act_warm[:, 0:4])
            nc.scalar.copy(act_warm[:, 4:8], act_warm[:, 0:4])

            # PE warmup: sustained matmuls on a gpsimd-built tile so the HAM
            # clock gate lifts before the real work arrives (no DMA needed).
            if WARMUP_MM:
                ident_f32 = cpool.tile([P, P], f32)
                ident_bf = cpool.tile([P, P], bf16)
                make_identity(nc, ident_f32[:])
                nc.vector.tensor_copy(ident_bf[:], ident_f32[:])
                warm_ps = ppool.tile([P, P], f32, tag="warm", bufs=1,
                                     name="warm_ps")
                for i in range(WARMUP_MM):
                    nc.tensor.matmul(
                        warm_ps[:], ident_bf[:], ident_bf[:],
                        start=True, stop=True,
                    )

            # ---- phase 1: S = b b^T, accumulated in PSUM ------------------
            # S rows split into two 128-row halves (separate PSUM tiles, and
            # separate evicting engines, so they share no dependencies).
            s0_ps = ppool.tile([P, C], f32, tag="acc", bufs=2, name="s0_ps")
            s1_ps = ppool.tile([P, C], f32, tag="acc", bufs=2, name="s1_ps")
            for m in range(MCH):
                bt = bt_sb[:, m, :]
                nc.tensor.matmul(s0_ps[:], bt_sb[:, m, 0:P], bt,
                                 start=(m == 0), stop=(m == MCH - 1))
                nc.tensor.matmul(s1_ps[:], bt_sb[:, m, P:C], bt,
                                 start=(m == 0), stop=(m == MCH - 1))
            s0_sb = bpool.tile([P, C], bf16)
            s1_sb = bpool.tile([P, C], bf16)
            nc.vector.tensor_copy(s0_sb[:], s0_ps[:])
            nc.scalar.copy(s1_sb[:], s1_ps[:])
            s_sb = (s0_sb, s1_sb)

            # ---- tail chain: M3 = S K2, R^T = phi_w M3 --------------------
            NBLK = HALF // RB
            ta_sb = bpool.tile([Ci, HALF], bf16)
            rt_sb = bpool.tile([Ci, C], bf16)

            def emit_ta(t):
                rows = slice(t * RB, (t + 1) * RB)
                ta_ps = ppool.tile([Ci, RB], f32, tag="ps", name=f"taps{t}")
                for cc in range(CC):
                    nc.tensor.matmul(ta_ps[:], thwT[:, cc, :],
                                     a_sb[:, cc, rows],
                                     start=(cc == 0), stop=(cc == CC - 1))
                if t % 2 == 0:
                    nc.scalar.activation(ta_sb[:, rows], ta_ps[:],
                                         Act.Identity, bias=thb_sb)
                else:
                    nc.vector.tensor_tensor(
                        ta_sb[:, rows], ta_ps[:],
                        thb_sb.broadcast_to([Ci, RB]),
                        op=mybir.AluOpType.add)

            # M3[c1, co] = sum_c2 S[c1, c2] K2[c2, co]; the stationary
            # operand S[c2, c1-half] is the stored S tile by symmetry.
            emit_ta(0)
            m3_sbs = []
            for hc1 in range(2):
                m3_ps = ppool.tile([P, C], f32, tag="ps", name=f"m3ps{hc1}")
                for hc2 in range(2):
                    nc.tensor.matmul(
                        m3_ps[:], s_sb[hc2][:, hc1 * P : (hc1 + 1) * P],
                        k2_sb[:, hc2, :],
                        start=(hc2 == 0), stop=(hc2 == 1),
                    )
                m3_sb = bpool.tile([P, C], bf16, name=f"m3sb{hc1}")
                if hc1 == 0:
                    nc.vector.tensor_copy(m3_sb[:], m3_ps[:])
                else:
                    nc.scalar.copy(m3_sb[:], m3_ps[:])
                m3_sbs.append(m3_sb)
            emit_ta(1)
            rt_ps = ppool.tile([Ci, C], f32, tag="ps", name="rt_ps")
            for h in range(2):
                nc.tensor.matmul(rt_ps[:], phwT[:, h, :], m3_sbs[h][:],
                                 start=(h == 0), stop=(h == 1))
            nc.vector.tensor_tensor(rt_sb[:], rt_ps[:], rtc_sb,
                                    op=mybir.AluOpType.add)
            emit_ta(2)
            emit_ta(3)

            # ---- out = R^T-contract ta, BN shift, store -------------------
            for r in range(NBLK):
                rows = slice(r * RB, (r + 1) * RB)
                osb = wpool.tile([P, CC, RB], bf16, tag="osb", bufs=4,
                                 name=f"osb{r}")
                for co in range(CC):
                    o_ps = ppool.tile([P, RB], f32, tag="ps",
                                      name=f"ops{r}{co}")
                    nc.tensor.matmul(o_ps[:], rt_sb[:, co * P : (co + 1) * P],
                                     ta_sb[:, rows], start=True, stop=True)
                    if co == 0:
                        nc.scalar.activation(osb[:, 0, :], o_ps[:],
                                             Act.Identity,
                                             bias=shift_in[:, 0:1])
                    else:
                        nc.vector.tensor_tensor(
                            osb[:, 1, :], o_ps[:],
                            shift_in[:, 1:2].broadcast_to([P, RB]),
                            op=mybir.AluOpType.add,
                        )
                nc.sync.dma_start(
                    out=out_d[:, :, rows].rearrange("c p r -> p c r"),
                    in_=osb[:],
                )

    nc.compile()
    return nc


def _get_nc():
    if "nc" not in _CACHE:
        _CACHE["nc"] = _build()
    return _CACHE["nc"]


def _prep_in_maps(a, b, theta_w, theta_b, phi_w, phi_b, g_w, g_b, W_w,
                  bn_gamma, bn_beta, bn_mean, bn_var):
    import ml_dtypes

    f = np.float32
    bf = ml_dtypes.bfloat16
    a4 = np.asarray(a, f).reshape(B, C, N_PIX)
    b4 = np.asarray(b, f).reshape(B, C, N_PIX)
    theta_w = np.asarray(theta_w, f)
    phi_w = np.asarray(phi_w, f)
    g_w = np.asarray(g_w, f)
    W_w = np.asarray(W_w, f)
    theta_b = np.asarray(theta_b, f)
    phi_b = np.asarray(phi_b, f)
    g_b = np.asarray(g_b, f)

    scale = (np.asarray(bn_gamma, f)
             / np.sqrt(np.asarray(bn_var, f) + BN_EPS)).astype(f)
    shift = (np.asarray(bn_beta, f) - np.asarray(bn_mean, f) * scale).astype(f)
    inv_n = 1.0 / np.float64(N_PIX)
    WT = (W_w * scale[:, None]).T                # [ci2, c_out]

    wpack = np.zeros((P, WCOLS), f)
    wpack[:, 0:128] = phi_w.T[0:P]
    wpack[:, 128:256] = phi_w.T[P:C]
    K2 = g_w.T @ WT                              # [c2, c_out]
    wpack[:, 256:512] = K2[0:P]
    wpack[:, 512:768] = K2[P:C]
    thT = (theta_w * inv_n).T                    # [C, Ci]
    wpack[:, 768:896] = thT[0:P]
    wpack[:, 896:1024] = thT[P:C]
    wpack = np.ascontiguousarray(wpack.astype(bf))

    # R^T bias correction from rowsums of b (exact; zero for zero biases)
    rsb = b4.sum(axis=2)                        # [B, C]
    s_phi = rsb @ phi_w.T                       # [B, Ci]
    s_g = rsb @ g_w.T                           # [B, Ci]
    qp = HALF // NTA

    in_maps = []
    for core in range(N_CORES):
        s, h = divmod(core, 2)
        cmi = (phi_b[:, None] * s_g[s][None, :]
               + s_phi[s][:, None] * g_b[None, :]
               + N_PIX * phi_b[:, None] * g_b[None, :]).astype(f)
        rtc = cmi @ WT                          # [ci1, c_out]
        vpack = np.zeros((P, VCOLS), f)
        vpack[:, 0] = shift[:P]
        vpack[:, 1] = shift[P:]
        vpack[:, 2] = theta_b * inv_n
        vpack[:, 3:259] = rtc
        ah = a4[s][:, h * HALF : (h + 1) * HALF]
        in_maps.append({
            "bT": np.ascontiguousarray(
                b4[s].T.reshape(NCH, 4, P, C).astype(bf)),
            "wpack": wpack,
            "vpack": np.ascontiguousarray(vpack),
            "a_half": np.ascontiguousarray(
                ah.reshape(CC, P, NTA, qp).transpose(2, 0, 1, 3).astype(bf)),
        })
    return in_maps


def run(inputs: dict, trace: bool = False):
    from concourse.bass_utils import run_bass_kernel_spmd

    nc = _get_nc()
    in_maps = _prep_in_maps(**inputs)
    res = run_bass_kernel_spmd(nc, in_maps, list(range(N_CORES)), trace=trace)
    out = np.empty((B, C, N_PIX), np.float32)
    for core in range(N_CORES):
        s, h = divmod(core, 2)
        out[s][:, h * HALF : (h + 1) * HALF] = \
            res.results[core]["out"].reshape(C, HALF).astype(np.float32)
    return out.reshape(B, C, H, W), res


def kernel(**inputs) -> np.ndarray:
    out, _ = run(inputs, trace=False)
    return out
